# revision 5
# baseline (speedup 1.0000x reference)
"""ChebyKAN linear layer on 8 Trainium2 NeuronCores.

Math: y[b,o] = sum_{i,d} T_d(w[b,i]) * C[i,o,d], with w = tanh(tanh(x)) and
T_d the Chebyshev polynomials (cos(d*arccos(w)) == T_d(w) for |w|<=1).

The ACT engine has no arccos/cos, so the device evaluates the basis as plain
monomials w^j (built from ACT Square + DVE multiplies); the Chebyshev->monomial
basis change is folded into the coefficients on the host (exact 9x9 integer
matrix, applied in float64). The j=0 term is a constant => folded into a per-o
bias added during PSUM evacuation.

Sharding: data-parallel over batch b (16384 -> 2048/core); coeffs replicated.
x is pre-transposed on the host so the contraction dim (c_in) lands on SBUF
partitions; the kernel computes y^T per core and the host transposes back.
Matmuls run in float32r (full fp32 data, 1 cycle/row at N=512).
"""

import sys

if "/opt/trn_rl_repo" not in sys.path:
    sys.path.append("/opt/trn_rl_repo")

import numpy as np

import concourse.bacc as bacc
import concourse.tile as tile
from concourse import mybir
from concourse.bass_utils import run_bass_kernel_spmd

DEGREE = 8
B, C_IN, C_OUT = 16384, 512, 512
N_CORES = 8
NB = B // N_CORES            # 2048 batch rows per core
B_TILE = 512                 # batch window per PSUM accumulation phase
N_PHASES = NB // B_TILE      # 4
N_IB = C_IN // 128           # 4 contraction row-blocks
N_J = DEGREE                 # powers j=1..8 (j=0 folded into bias)
F32 = mybir.dt.float32
F32R = mybir.dt.float32r

_CACHE = {}


def _build():
    nc = bacc.Bacc("TRN2", target_bir_lowering=False, debug=False)
    xt = nc.dram_tensor("xt", [C_IN, NB], F32, kind="ExternalInput")
    wmat = nc.dram_tensor(
        "wmat", [N_IB * N_J * 128, C_OUT], F32R, kind="ExternalInput"
    )
    biasv = nc.dram_tensor("biasv", [128, 4], F32, kind="ExternalInput")
    yt = nc.dram_tensor("yt", [C_OUT, NB], F32, kind="ExternalOutput")

    Tanh = mybir.ActivationFunctionType.Tanh
    Square = mybir.ActivationFunctionType.Square
    Identity = mybir.ActivationFunctionType.Identity

    with tile.TileContext(nc) as tc:
        with (
            tc.tile_pool(name="const", bufs=1) as const_pool,
            tc.tile_pool(name="wts", bufs=1) as wpool,
            tc.tile_pool(name="acts", bufs=1) as xpool,
            tc.tile_pool(name="pows", bufs=3) as ppool,
            tc.tile_pool(name="outs", bufs=2) as opool,
            tc.tile_pool(name="psum", bufs=2, space="PSUM") as pspool,
        ):
            bias_t = const_pool.tile([128, 4], F32)
            nc.sync.dma_start(out=bias_t[:], in_=biasv.ap())

            # coefficient chunks: w_sb[ib][:, j, :] is [128 i_inner, 512 o]
            w_sb = []
            for ib in range(N_IB):
                wt = wpool.tile([128, N_J, C_OUT], F32R, tag=f"w{ib}")
                nc.sync.dma_start(
                    out=wt[:],
                    in_=wmat.ap()[ib * N_J * 128 : (ib + 1) * N_J * 128, :].rearrange(
                        "(j p) o -> p j o", p=128
                    ),
                )
                w_sb.append(wt)

            wf = [None] * N_IB  # wf[ib]: [128 i, 2048 b] = tanh(tanh(x^T))
            for ph in range(N_PHASES):
                ps = [
                    pspool.tile([128, B_TILE], F32, tag=f"ps{oc}", name=f"ps{oc}_{ph}")
                    for oc in range(4)
                ]
                bsl = slice(ph * B_TILE, (ph + 1) * B_TILE)
                for ib in range(N_IB):
                    if ph == 0:
                        xt_t = xpool.tile([128, NB], F32, tag="xload", bufs=2)
                        nc.sync.dma_start(
                            out=xt_t[:], in_=xt.ap()[ib * 128 : (ib + 1) * 128, :]
                        )
                        nc.scalar.activation(xt_t[:], xt_t[:], Tanh)
                        wr_t = xpool.tile([128, NB], F32R, tag=f"wr{ib}", bufs=1)
                        nc.scalar.activation(wr_t[:], xt_t[:], Tanh)
                        wf[ib] = wr_t
                    w1 = wf[ib][:, bsl]
                    p2 = ppool.tile([128, B_TILE], F32R, tag="p2")
                    p3 = ppool.tile([128, B_TILE], F32R, tag="p3")
                    p4 = ppool.tile([128, B_TILE], F32R, tag="p4")
                    p5 = ppool.tile([128, B_TILE], F32R, tag="p5")
                    p6 = ppool.tile([128, B_TILE], F32R, tag="p6")
                    p7 = ppool.tile([128, B_TILE], F32R, tag="p7")
                    p8 = ppool.tile([128, B_TILE], F32R, tag="p8")
                    nc.scalar.activation(p2[:], w1, Square)
                    nc.vector.tensor_mul(p3[:], w1, p2[:])
                    nc.scalar.activation(p4[:], p2[:], Square)
                    nc.vector.tensor_mul(p5[:], w1, p4[:])
                    nc.scalar.activation(p6[:], p3[:], Square)
                    nc.vector.tensor_mul(p7[:], p3[:], p4[:])
                    nc.scalar.activation(p8[:], p4[:], Square)
                    chunks = [w1, p2[:], p3[:], p4[:], p5[:], p6[:], p7[:], p8[:]]
                    for j, ch in enumerate(chunks):
                        for oc in range(4):
                            nc.tensor.matmul(
                                ps[oc][:],
                                lhsT=w_sb[ib][:, j, oc * 128 : (oc + 1) * 128],
                                rhs=ch,
                                start=(ib == 0 and j == 0),
                                stop=(ib == N_IB - 1 and j == N_J - 1),
                            )
                for oc in range(4):
                    osb = opool.tile([128, B_TILE], F32, tag=f"osb{oc}")
                    nc.scalar.activation(
                        osb[:], ps[oc][:], Identity, bias=bias_t[:, oc : oc + 1]
                    )
                    nc.sync.dma_start(
                        out=yt.ap()[oc * 128 : (oc + 1) * 128, bsl], in_=osb[:]
                    )
    nc.compile()
    return nc


def _host_transform(cheby_coeffs):
    # T_d(w) = sum_j M[d, j] w^j  (exact integer recurrence, in float64)
    M = np.zeros((DEGREE + 1, DEGREE + 1))
    M[0, 0] = 1.0
    M[1, 1] = 1.0
    for k in range(1, DEGREE):
        M[k + 1, :] = -M[k - 1, :]
        M[k + 1, 1:] += 2.0 * M[k, :-1]
    Cp = np.einsum("iod,dj->ioj", cheby_coeffs.astype(np.float64), M)
    bias = Cp[:, :, 0].sum(axis=0)                       # [o]
    Wd = (
        Cp[:, :, 1:]
        .reshape(N_IB, 128, C_OUT, N_J)
        .transpose(0, 3, 1, 2)                           # [ib, j, i_inner, o]
        .reshape(N_IB * N_J * 128, C_OUT)
        .astype(np.float32)
    )
    bias_dev = np.ascontiguousarray(bias.reshape(4, 128).T.astype(np.float32))
    return np.ascontiguousarray(Wd), bias_dev


def kernel(x, cheby_coeffs):
    x = np.asarray(x, dtype=np.float32)
    cheby_coeffs = np.asarray(cheby_coeffs, dtype=np.float32)
    if "nc" not in _CACHE:
        _CACHE["nc"] = _build()
    nc = _CACHE["nc"]

    Wd, bias_dev = _host_transform(cheby_coeffs)
    xT = np.ascontiguousarray(x.T)                       # [c_in, b]
    in_maps = [
        {
            "xt": np.ascontiguousarray(xT[:, c * NB : (c + 1) * NB]),
            "wmat": Wd,
            "biasv": bias_dev,
        }
        for c in range(N_CORES)
    ]
    res = run_bass_kernel_spmd(nc, in_maps, core_ids=list(range(N_CORES)))
    y = np.concatenate([res.results[c]["yt"].T for c in range(N_CORES)], axis=0)
    return y


# revision 6
# speedup vs baseline: 1.1733x; 1.1733x over previous
"""ChebyKAN linear layer on 8 Trainium2 NeuronCores.

Math: y[b,o] = sum_{i,d} T_d(w[b,i]) * C[i,o,d], with w = tanh(tanh(x)) and
T_d the Chebyshev polynomials (cos(d*arccos(w)) == T_d(w) for |w|<=1).

The ACT engine has no arccos/cos, so the device evaluates the basis as plain
monomials w^j (built from ACT Square + DVE multiplies); the Chebyshev->monomial
basis change is folded into the coefficients on the host (exact 9x9 integer
matrix, applied in float64). The j=0 term is a constant => folded into a per-o
bias added during PSUM evacuation.

Sharding: data-parallel over batch b (16384 -> 2048/core); coeffs replicated.
x is pre-transposed on the host so the contraction dim (c_in) lands on SBUF
partitions; the kernel computes y^T per core and the host transposes back.

Matmuls run in float32r (1 cycle/row at N=512, vs 4 for plain fp32). The
power-chain intermediates stay full fp32; each matmul operand is rounded to
f32r exactly once (compounding f32r roundings through the squaring chain
costs ~15x in accuracy). Coefficients stream over the SWDGE (gpsimd) DMA ring
so the activation loads on the sync ring aren't queued behind them.
"""

import sys

if "/opt/trn_rl_repo" not in sys.path:
    sys.path.append("/opt/trn_rl_repo")

import numpy as np

import concourse.bacc as bacc
import concourse.tile as tile
from concourse import mybir
from concourse.bass_utils import run_bass_kernel_spmd

DEGREE = 8
B, C_IN, C_OUT = 16384, 512, 512
N_CORES = 8
NB = B // N_CORES            # 2048 batch rows per core
B_TILE = 512                 # batch window per PSUM accumulation phase
N_PHASES = NB // B_TILE      # 4
N_IB = C_IN // 128           # 4 contraction row-blocks
N_J = DEGREE                 # powers j=1..8 (j=0 folded into bias)
F32 = mybir.dt.float32
F32R = mybir.dt.float32r

_CACHE = {}


def _build():
    nc = bacc.Bacc("TRN2", target_bir_lowering=False, debug=False)
    xt = nc.dram_tensor("xt", [C_IN, NB], F32, kind="ExternalInput")
    wmat = nc.dram_tensor("wmat", [C_IN, N_J * C_OUT], F32R, kind="ExternalInput")
    biasv = nc.dram_tensor("biasv", [128, 4], F32, kind="ExternalInput")
    yt = nc.dram_tensor("yt", [C_OUT, NB], F32, kind="ExternalOutput")

    Tanh = mybir.ActivationFunctionType.Tanh
    Square = mybir.ActivationFunctionType.Square
    Identity = mybir.ActivationFunctionType.Identity

    with tile.TileContext(nc) as tc:
        with (
            tc.tile_pool(name="const", bufs=1) as const_pool,
            tc.tile_pool(name="wts", bufs=1) as wpool,
            tc.tile_pool(name="pows", bufs=3) as ppool,
            tc.tile_pool(name="outs", bufs=2) as opool,
            tc.tile_pool(name="psum", bufs=2, space="PSUM") as pspool,
        ):
            bias_t = const_pool.tile([128, 4], F32)
            nc.sync.dma_start(out=bias_t[:], in_=biasv.ap())

            # coefficient chunks, [128 i_inner, 512 o] each, via the SWDGE ring
            w_sb = {}
            for ib in range(N_IB):
                for j in range(N_J):
                    wc = wpool.tile(
                        [128, C_OUT], F32R, tag=f"wc{ib}_{j}", name=f"wc{ib}_{j}"
                    )
                    nc.gpsimd.dma_start(
                        out=wc[:],
                        in_=wmat.ap()[
                            ib * 128 : (ib + 1) * 128,
                            j * C_OUT : (j + 1) * C_OUT,
                        ],
                    )
                    w_sb[ib, j] = wc

            for ph in range(N_PHASES):
                ps = [
                    pspool.tile([128, B_TILE], F32, tag=f"ps{oc}", name=f"ps{oc}_{ph}")
                    for oc in range(4)
                ]
                bsl = slice(ph * B_TILE, (ph + 1) * B_TILE)
                for ib in range(N_IB):
                    xl = ppool.tile([128, B_TILE], F32, tag="xl")
                    nc.sync.dma_start(
                        out=xl[:], in_=xt.ap()[ib * 128 : (ib + 1) * 128, bsl]
                    )
                    nc.scalar.activation(xl[:], xl[:], Tanh)
                    w1 = ppool.tile([128, B_TILE], F32, tag="w1")
                    nc.scalar.activation(w1[:], xl[:], Tanh)
                    # full-fp32 chain nodes
                    p2 = ppool.tile([128, B_TILE], F32, tag="p2")
                    p3 = ppool.tile([128, B_TILE], F32, tag="p3")
                    p4 = ppool.tile([128, B_TILE], F32, tag="p4")
                    nc.scalar.activation(p2[:], w1[:], Square)
                    nc.vector.tensor_mul(p3[:], w1[:], p2[:])
                    nc.scalar.activation(p4[:], p2[:], Square)
                    # f32r-rounded matmul operands (one rounding each)
                    w1r = ppool.tile([128, B_TILE], F32R, tag="w1r")
                    p2r = ppool.tile([128, B_TILE], F32R, tag="p2r")
                    p3r = ppool.tile([128, B_TILE], F32R, tag="p3r")
                    p4r = ppool.tile([128, B_TILE], F32R, tag="p4r")
                    p5 = ppool.tile([128, B_TILE], F32R, tag="p5")
                    p6 = ppool.tile([128, B_TILE], F32R, tag="p6")
                    p7 = ppool.tile([128, B_TILE], F32R, tag="p7")
                    p8 = ppool.tile([128, B_TILE], F32R, tag="p8")
                    nc.vector.tensor_copy(w1r[:], w1[:])
                    nc.vector.tensor_copy(p2r[:], p2[:])
                    nc.vector.tensor_copy(p3r[:], p3[:])
                    nc.vector.tensor_copy(p4r[:], p4[:])
                    nc.vector.tensor_mul(p5[:], w1[:], p4[:])
                    nc.scalar.activation(p6[:], p3[:], Square)
                    nc.vector.tensor_mul(p7[:], p3[:], p4[:])
                    nc.scalar.activation(p8[:], p4[:], Square)
                    chunks = [w1r, p2r, p3r, p4r, p5, p6, p7, p8]
                    for j, ch in enumerate(chunks):
                        for oc in range(4):
                            nc.tensor.matmul(
                                ps[oc][:],
                                lhsT=w_sb[ib, j][:, oc * 128 : (oc + 1) * 128],
                                rhs=ch[:],
                                start=(ib == 0 and j == 0),
                                stop=(ib == N_IB - 1 and j == N_J - 1),
                            )
                for oc in range(4):
                    osb = opool.tile([128, B_TILE], F32, tag=f"osb{oc}", name=f"osb{oc}")
                    nc.scalar.activation(
                        osb[:], ps[oc][:], Identity, bias=bias_t[:, oc : oc + 1]
                    )
                    nc.sync.dma_start(
                        out=yt.ap()[oc * 128 : (oc + 1) * 128, bsl], in_=osb[:]
                    )
    nc.compile()
    return nc


def _host_transform(cheby_coeffs):
    # T_d(w) = sum_j M[d, j] w^j  (exact integer recurrence, in float64)
    M = np.zeros((DEGREE + 1, DEGREE + 1))
    M[0, 0] = 1.0
    M[1, 1] = 1.0
    for k in range(1, DEGREE):
        M[k + 1, :] = -M[k - 1, :]
        M[k + 1, 1:] += 2.0 * M[k, :-1]
    Cp = np.einsum("iod,dj->ioj", cheby_coeffs.astype(np.float64), M)
    bias = Cp[:, :, 0].sum(axis=0)                       # [o]
    # [i, j*512+o]: per-partition-contiguous coefficient rows
    Wd = np.ascontiguousarray(
        Cp[:, :, 1:].transpose(0, 2, 1).reshape(C_IN, N_J * C_OUT).astype(np.float32)
    )
    bias_dev = np.ascontiguousarray(bias.reshape(4, 128).T.astype(np.float32))
    return Wd, bias_dev


def kernel(x, cheby_coeffs):
    x = np.asarray(x, dtype=np.float32)
    cheby_coeffs = np.asarray(cheby_coeffs, dtype=np.float32)
    if "nc" not in _CACHE:
        _CACHE["nc"] = _build()
    nc = _CACHE["nc"]

    Wd, bias_dev = _host_transform(cheby_coeffs)
    xT = np.ascontiguousarray(x.T)                       # [c_in, b]
    in_maps = [
        {
            "xt": np.ascontiguousarray(xT[:, c * NB : (c + 1) * NB]),
            "wmat": Wd,
            "biasv": bias_dev,
        }
        for c in range(N_CORES)
    ]
    res = run_bass_kernel_spmd(nc, in_maps, core_ids=list(range(N_CORES)))
    y = np.concatenate([res.results[c]["yt"].T for c in range(N_CORES)], axis=0)
    return y
